# revision 26
# baseline (speedup 1.0000x reference)
"""Multi-head self-attention TRN2 Bass kernel (8-core SPMD).

Problem: x[2,2048,1024] -> qkv proj (w_qkv[1024,3072]) -> 16-head attention
-> out proj (w_out[1024,1024] + b_out) -> [2,2048,1024], all fp32.

Sharding: core i handles batch b=i//4 and head-group g=i%4 (4 heads each).
Each core computes a partial out-projection (its 256 rows of w_out); the
host sums the 4 partials per batch and adds the bias.

Per-core kernel layout strategy:
  - x arrives transposed from host as xT [c, tokens] fp16; DMAs split
    across the two HWDGE queues (sync + scalar), first-quarter columns
    first, so the first qkv chain starts after ~1.5MB instead of 4.5MB.
  - w_qkv slices arrive p-major (head-pair-major) so each head-pair's
    projection gates on 256KB.
  - qT/kT computed per head-pair as [128=2*64 d, 2048 tokens]; the k
    token-slices interleave with the first attention unit so the scalar
    engine (exp) starts as early as possible.
  - v computed in natural [keys, d] layout, 4 heads wide, stored with a
    constant-1.0 column block per head so the attention O^T matmul also
    produces the softmax denominators (partitions 64-127).
  - Attention per (head-pair p, q-slice of 512): for each key chunk kc,
    the TWO heads' score matmuls (K=64 row-tiles at partitions 0-63 /
    64-127) write adjacent 512-col halves of ONE PSUM tile [128,1024].
    Both matmuls gate on the same pool buffer, so the tile scheduler
    keeps them adjacent and the PE runs them CONCURRENTLY (row tiling,
    2x). One ACT exp (scale=1/8 fused) evicts the pair to SBUF fp16.
    O^T accumulates over 16 key-chunks with lhsT = v_aug.
    Normalize: DVE reciprocal of the sums rows, DVE multiply.
  - The last unit's O accumulator lives in the (by then idle) st pool so
    the final two units' O chains overlap instead of serializing on the
    small-psum pool.
  - Out projection partials written back fp16, summed on host in fp32.

Matmuls in fp16 (1 cycle/row on TRN2).
"""

import sys

if "/opt/trn_rl_repo" not in sys.path:
    sys.path.insert(0, "/opt/trn_rl_repo")

import numpy as np

import concourse.bacc as bacc
import concourse.mybir as mybir
import concourse.tile as tile
from concourse.bass_utils import run_bass_kernel_spmd

F32 = mybir.dt.float32
F16 = mybir.dt.float16

N_TOK = 2048
C = 1024
D = 64
CC = C // 128           # 8 contraction chunks
QS = N_TOK // 512       # 4 q-slices
KC = N_TOK // 128       # 16 key chunks

_COMPILED = None

MM_DT = F16


def build_nc(mm_dt=None):
    mm_dt = MM_DT if mm_dt is None else mm_dt
    nc = bacc.Bacc("TRN2", target_bir_lowering=False)

    xt_d = nc.declare_dram_parameter("xt", [C, N_TOK], mm_dt, isOutput=False)
    wq_d = nc.declare_dram_parameter("wq", [128, 2 * CC * 128], mm_dt, isOutput=False)
    wk_d = nc.declare_dram_parameter("wk", [128, 2 * CC * 128], mm_dt, isOutput=False)
    wv_d = nc.declare_dram_parameter("wv", [128, CC * 256], mm_dt, isOutput=False)
    wo_d = nc.declare_dram_parameter("wo", [128, 2 * C], mm_dt, isOutput=False)
    out_d = nc.declare_dram_parameter("out", [N_TOK, C], F16, isOutput=True)

    HP = CC * 128  # columns per head-pair in p-major weight layout

    with tile.TileContext(nc) as tc:
        with (
            tc.tile_pool(name="const", bufs=1) as const_pool,
            tc.tile_pool(name="qkt", bufs=1) as qkt_pool,
            tc.tile_pool(name="vsb", bufs=1) as v_pool,
            tc.tile_pool(name="otsb", bufs=1) as ot_sb_pool,
            tc.tile_pool(name="wop", bufs=1) as wo_pool,
            tc.tile_pool(name="outsb", bufs=2) as out_sb_pool,
            tc.tile_pool(name="pss", bufs=2, space="PSUM") as ps_small,
            tc.tile_pool(name="xTp", bufs=1) as xt_pool,
            tc.tile_pool(name="w3", bufs=3) as w3_pool,
            tc.tile_pool(name="stp", bufs=3, space="PSUM") as st_psum,
            tc.tile_pool(name="pt", bufs=38) as pt_pool,
            tc.tile_pool(name="rcp", bufs=2) as rcp_pool,
        ):
            wq_sb = w3_pool.tile([128, 2, CC, 128], mm_dt, tag="w3")
            wk_sb = w3_pool.tile([128, 2, CC, 128], mm_dt, tag="w3")
            wv_sb = w3_pool.tile([128, CC, 256], mm_dt, tag="w3")
            wo_sb = wo_pool.tile([128, 2, C], mm_dt, tag="wo")
            xT = xt_pool.tile([128, CC, N_TOK], mm_dt, tag="xT")

            # Input DMAs first, split across the two HWDGE queues (SP + ACT),
            # packed into few large transfers (engine DMA-issue costs ~600ns
            # each). The first k/q chains gate on the p=0 weight slices plus
            # the first-quarter token columns of every contraction chunk.
            wk_v = wk_sb[:].rearrange("p a c n -> p (a c n)")
            wq_v = wq_sb[:].rearrange("p a c n -> p (a c n)")
            nc.sync.dma_start(out=wk_v[:, 0:HP], in_=wk_d[:, 0:HP])
            nc.scalar.dma_start(out=wq_v[:, 0:HP], in_=wq_d[:, 0:HP])
            for cc in range(CC):
                eng = nc.sync if cc % 2 == 0 else nc.scalar
                eng.dma_start(out=xT[:, cc, 0:512], in_=xt_d[cc * 128:(cc + 1) * 128, 0:512])
            for cc in range(CC):
                eng = nc.sync if cc % 2 == 1 else nc.scalar
                eng.dma_start(out=xT[:, cc, 512:N_TOK], in_=xt_d[cc * 128:(cc + 1) * 128, 512:N_TOK])
            nc.sync.dma_start(out=wk_v[:, HP:2 * HP], in_=wk_d[:, HP:2 * HP])
            nc.scalar.dma_start(out=wq_v[:, HP:2 * HP], in_=wq_d[:, HP:2 * HP])
            nc.sync.dma_start(out=wv_sb[:].rearrange("p c n -> p (c n)"), in_=wv_d[:])
            nc.scalar.dma_start(out=wo_sb[:].rearrange("p c n -> p (c n)"), in_=wo_d[:])

            ones256 = const_pool.tile([128, 256], mm_dt, tag="ones256")
            nc.vector.memset(ones256[:], 1.0)
            # touch Exp early so the ACT table set loads during phase 1.
            actwarm = const_pool.tile([1, 1], F32, tag="actwarm")
            nc.scalar.activation(
                actwarm[:], ones256[0:1, 0:1], mybir.ActivationFunctionType.Exp
            )
            # PE warmup: no-dependency matmuls ramp the tensor-engine clock
            # (0.65 -> 2.4 GHz takes ~3us of busy) while input DMAs land, so
            # the first real qkv chains run at full speed.
            warm = st_psum.tile([128, 1024], F32, tag="st", name="warm")
            for i in range(12):
                nc.tensor.matmul(
                    warm[:, 0:256],
                    ones256[:, 0:128],
                    ones256[:],
                    start=True,
                    stop=True,
                )

            qT = [qkt_pool.tile([128, N_TOK], mm_dt, tag=f"qT{p}", name=f"qT{p}") for p in range(2)]
            kT = [qkt_pool.tile([128, N_TOK], mm_dt, tag=f"kT{p}", name=f"kT{p}") for p in range(2)]
            oT = [ot_sb_pool.tile([128, N_TOK], mm_dt, tag=f"oT{p}", name=f"oT{p}") for p in range(2)]
            # v_aug: per key-chunk, per head: [64 v columns | 64 ones columns]
            # -> the O^T matmul (M=128) emits softmax denominators on
            # partitions 64..127, already broadcast.
            v_sb = v_pool.tile([128, KC, 4 * 128], mm_dt, tag="v")
            v_aug_view = v_sb[:].rearrange("p k (h e) -> p k h e", h=4)

            def emit_k_ts(p, ts):
                sl = slice(ts * 512, (ts + 1) * 512)
                acc = ps_small.tile([128, 512], F32, tag="ps", name="acc")
                for cc in range(CC):
                    nc.tensor.matmul(
                        acc[:],
                        wk_sb[:, p, cc, :],
                        xT[:, cc, sl],
                        start=(cc == 0),
                        stop=(cc == CC - 1),
                    )
                nc.vector.tensor_copy(kT[p][:, sl], acc[:])

            def emit_q(p, ts):
                sl = slice(ts * 512, (ts + 1) * 512)
                acc = ps_small.tile([128, 512], F32, tag="ps", name="acc")
                for cc in range(CC):
                    nc.tensor.matmul(
                        acc[:],
                        wq_sb[:, p, cc, :],
                        xT[:, cc, sl],
                        start=(cc == 0),
                        stop=(cc == CC - 1),
                    )
                nc.vector.tensor_copy(qT[p][:, sl], acc[:])

            def emit_v(lo, hi):
                for kc in range(lo, hi):
                    acc = ps_small.tile([128, 256], F32, tag="ps", name="acc")
                    for cc in range(CC):
                        nc.tensor.matmul(
                            acc[:],
                            xT[:, cc, kc * 128:(kc + 1) * 128],
                            wv_sb[:, cc, :],
                            start=(cc == 0),
                            stop=(cc == CC - 1),
                        )
                    nc.vector.tensor_copy(
                        v_aug_view[:, kc, :, 0:64],
                        acc[:].rearrange("p (h e) -> p h e", h=4),
                    )

            pts_map = {}
            ots_map = {}

            def emit_sexp(ts, p, k0, k1):
                # Per key-chunk: both heads' score matmuls (K=64 row tiles at
                # partitions 0-63 / 64-127) write adjacent halves of one PSUM
                # tile -> PE runs them concurrently. Bursts of 2 kc reduce
                # PE array mode switches (64-row tiling <-> full 128).
                sl = slice(ts * 512, (ts + 1) * 512)
                unit = pts_map.setdefault((ts, p), {})
                for kc0 in range(k0, k1, 2):
                    sts = []
                    for kc in (kc0, kc0 + 1):
                        st = st_psum.tile([128, 1024], F32, tag="st", name="st")
                        for h in range(2):
                            hp = slice(h * 64, (h + 1) * 64)
                            nc.tensor.matmul(
                                st[:, h * 512:(h + 1) * 512],
                                kT[p][hp, kc * 128:(kc + 1) * 128],
                                qT[p][hp, sl],
                                start=True,
                                stop=True,
                            )
                        sts.append(st)
                    for st, kc in zip(sts, (kc0, kc0 + 1)):
                        pt = pt_pool.tile([128, 1024], mm_dt, tag="pt", name="pt")
                        nc.scalar.activation(
                            pt[:],
                            st[:],
                            mybir.ActivationFunctionType.Exp,
                            scale=0.125,
                        )
                        unit[kc] = pt

            def emit_o(ts, p, k0, k1, act_norm=False):
                unit = pts_map[(ts, p)]
                if (ts, p) not in ots_map:
                    ots_map[(ts, p)] = [
                        ps_small.tile([128, 512], F32, tag="ps", name=f"ot{h}")[:]
                        for h in range(2)
                    ]
                ots = ots_map[(ts, p)]
                for kc in range(k0, k1):
                    pt = unit.pop(kc)
                    for h in range(2):
                        a = 2 * p + h
                        nc.tensor.matmul(
                            ots[h],
                            v_sb[:, kc, a * 128:(a + 1) * 128],
                            pt[:, h * 512:(h + 1) * 512],
                            start=(kc == 0),
                            stop=(kc == KC - 1),
                        )
                if k1 < KC:
                    return
                sl = slice(ts * 512, (ts + 1) * 512)
                numer = rcp_pool.tile([128, 512], F32, tag="numer")
                sums = rcp_pool.tile([128, 512], F32, tag="sums")
                for h in range(2):
                    hq = slice(h * 64, (h + 1) * 64)
                    nc.vector.tensor_copy(numer[hq, :], ots[h][0:64, :])
                    if act_norm:
                        # last unit: scalar engine is idle by now; moving the
                        # sums eviction there shortens the tail's DVE chain.
                        nc.scalar.activation(
                            sums[hq, :], ots[h][64:128, :],
                            mybir.ActivationFunctionType.Copy,
                        )
                    else:
                        nc.vector.tensor_copy(sums[hq, :], ots[h][64:128, :])
                rcp = rcp_pool.tile([128, 512], F32, tag="rcp")
                scr = rcp_pool.tile([128, 512], F32, tag="scr")
                nc.vector.reciprocal_approx_accurate(
                    out=rcp[:], in_=sums[:], scratch=scr[:]
                )
                nc.vector.tensor_mul(oT[p][:, sl], numer[:], rcp[:])

            def emit_proj(t0, t1, split_dma=False):
                # proj accumulators live in the st tag so proj bursts do not
                # touch the ps rotation (which sequences the O accumulators).
                for t in range(t0, t1):
                    outp = out_sb_pool.tile([128, C], F16, tag="outp")
                    for ns in range(2):
                        po = st_psum.tile([128, 512], F32, tag="st", name="po")
                        for p in range(2):
                            nc.tensor.matmul(
                                po[:],
                                oT[p][:, t * 128:(t + 1) * 128],
                                wo_sb[:, p, ns * 512:(ns + 1) * 512],
                                start=(p == 0),
                                stop=(p == 1),
                            )
                        nc.vector.tensor_copy(outp[:, ns * 512:(ns + 1) * 512], po[:])
                    eng = nc.scalar if (split_dma and t % 2) else nc.sync
                    eng.dma_start(
                        out=out_d[t * 128:(t + 1) * 128, :], in_=outp[:]
                    )

            # Software-pipelined schedule. Unit order:
            #   u0..u7 = (0,0),(1,0),(2,0),(0,1),(1,1),(3,0),(2,1),(3,1)
            # Each unit window weaves 2-kc sexp batches (the ACT pacer) with
            # small slices of the previous unit's O accumulation and PE
            # fillers (v, k/q chains, proj t-chunks) so the PE keeps the
            # scalar engine fed instead of draining whole blocks first.
            # The last three units' O chase their own exps (the ps rotation
            # frees up once qkv accs are done) to shrink the tail.
            # opening + u0 = (0,0)
            emit_k_ts(0, 0)
            emit_q(0, 0)
            emit_sexp(0, 0, 0, 4)
            emit_k_ts(0, 1)
            emit_sexp(0, 0, 4, 8)
            emit_k_ts(0, 2)
            emit_sexp(0, 0, 8, 12)
            emit_k_ts(0, 3)
            emit_sexp(0, 0, 12, 14)
            emit_q(0, 1)
            for kc in range(KC):
                nc.vector.tensor_copy(
                    v_aug_view[:, kc, :, 64:128],
                    ones256[:].rearrange("p (h e) -> p h e", h=4),
                )
            emit_sexp(1, 0, 0, 4)               # handover u0 -> u1
            emit_sexp(0, 0, 14, 16); emit_v(0, 4)
            # u1 = (1,0)
            emit_sexp(1, 0, 4, 6); emit_v(4, 6)
            emit_sexp(1, 0, 6, 8); emit_v(6, 8)
            emit_sexp(1, 0, 8, 10); emit_v(8, 10)
            emit_sexp(1, 0, 10, 12); emit_v(10, 12)
            emit_sexp(1, 0, 12, 14); emit_v(12, 14); emit_v(14, 16); emit_q(0, 2)
            emit_sexp(2, 0, 0, 4)               # handover u1 -> u2
            emit_sexp(1, 0, 14, 16)
            # u2 = (2,0)
            emit_sexp(2, 0, 4, 6); emit_o(0, 0, 0, 4)
            emit_sexp(2, 0, 6, 8); emit_o(0, 0, 4, 8)
            emit_sexp(2, 0, 8, 10); emit_o(0, 0, 8, 12)
            emit_sexp(2, 0, 10, 12); emit_o(0, 0, 12, 16); emit_k_ts(1, 0)
            emit_sexp(2, 0, 12, 14); emit_q(1, 0); emit_k_ts(1, 1)
            emit_sexp(0, 1, 0, 4)               # handover u2 -> u3
            emit_sexp(2, 0, 14, 16); emit_k_ts(1, 2)
            # u3 = (0,1)
            emit_sexp(0, 1, 4, 6); emit_k_ts(1, 3)
            emit_sexp(0, 1, 6, 8); emit_q(1, 1)
            emit_sexp(0, 1, 8, 10); emit_o(1, 0, 0, 4)
            emit_sexp(0, 1, 10, 12); emit_o(1, 0, 4, 8)
            emit_sexp(0, 1, 12, 14); emit_o(1, 0, 8, 12); emit_o(1, 0, 12, 16)
            emit_sexp(1, 1, 0, 4)               # handover u3 -> u4
            emit_sexp(0, 1, 14, 16); emit_q(0, 3)
            # u4 = (1,1)
            emit_sexp(1, 1, 4, 6); emit_o(2, 0, 0, 4)
            emit_sexp(1, 1, 6, 8); emit_o(2, 0, 4, 8)
            emit_sexp(1, 1, 8, 10); emit_o(2, 0, 8, 12)
            emit_sexp(1, 1, 10, 12); emit_o(2, 0, 12, 16); emit_o(0, 1, 0, 6)
            emit_sexp(1, 1, 12, 14); emit_o(0, 1, 6, 12)
            emit_sexp(3, 0, 0, 4)               # handover u4 -> u5
            emit_sexp(1, 1, 14, 16); emit_o(0, 1, 12, 16)
            # u5 = (3,0)
            emit_sexp(3, 0, 4, 6); emit_q(1, 2)
            emit_sexp(3, 0, 6, 8); emit_proj(0, 2)
            emit_sexp(3, 0, 8, 10); emit_proj(2, 4)
            emit_sexp(3, 0, 10, 12); emit_o(1, 1, 0, 6)
            emit_sexp(3, 0, 12, 14); emit_o(1, 1, 6, 12); emit_o(1, 1, 12, 16)
            emit_sexp(2, 1, 0, 4)               # handover u5 -> u6
            emit_sexp(3, 0, 14, 16); emit_q(1, 3)
            # u6 = (2,1)
            emit_sexp(2, 1, 4, 6); emit_proj(4, 6)
            emit_sexp(2, 1, 6, 8); emit_proj(6, 8)
            emit_sexp(2, 1, 8, 10); emit_o(3, 0, 0, 6)
            emit_sexp(2, 1, 10, 12); emit_o(3, 0, 6, 12)
            emit_sexp(2, 1, 12, 14); emit_o(3, 0, 12, 16); emit_o(2, 1, 0, 4)
            emit_sexp(3, 1, 0, 4)               # handover u6 -> u7
            emit_sexp(2, 1, 14, 16); emit_o(2, 1, 4, 6)
            # u7 = (3,1)
            emit_sexp(3, 1, 4, 6); emit_o(2, 1, 6, 10)
            emit_sexp(3, 1, 6, 8); emit_o(2, 1, 10, 14)
            emit_sexp(3, 1, 8, 10); emit_o(2, 1, 14, 16); emit_proj(8, 10)
            emit_sexp(3, 1, 10, 12); emit_proj(10, 12); emit_o(3, 1, 0, 4)
            emit_sexp(3, 1, 12, 14); emit_o(3, 1, 4, 8)
            emit_sexp(3, 1, 14, 16); emit_o(3, 1, 8, 12)
            emit_o(3, 1, 12, 16)
            emit_proj(12, 16, split_dma=True)

    nc.compile()
    return nc


def _shard_inputs(x, w_qkv, w_out):
    xts = [np.ascontiguousarray(x[b].T).astype(np.float16) for b in range(2)]
    in_maps = []
    for i in range(8):
        b, g = divmod(i, 4)
        cs = slice(256 * g, 256 * (g + 1))

        def shuf(w):
            # [1024, 256] -> [128, CC * 256] with chunk-major columns
            n = w.shape[1]
            return np.ascontiguousarray(
                w.reshape(CC, 128, n).transpose(1, 0, 2).reshape(128, CC * n)
            ).astype(np.float16)

        def shuf_p(w):
            # [1024, 256] -> [128, 2 * CC * 128] head-pair-major
            return np.ascontiguousarray(
                w.reshape(CC, 128, 2, 128).transpose(1, 2, 0, 3).reshape(128, 2 * CC * 128)
            ).astype(np.float16)

        in_maps.append({
            "xt": xts[b],
            "wq": shuf_p(w_qkv[:, cs]),
            "wk": shuf_p(w_qkv[:, 1024 + 256 * g:1024 + 256 * (g + 1)]),
            "wv": shuf(w_qkv[:, 2048 + 256 * g:2048 + 256 * (g + 1)]),
            "wo": np.ascontiguousarray(
                w_out[cs, :].reshape(2, 128, 1024).transpose(1, 0, 2).reshape(128, 2048)
            ).astype(np.float16),
        })
    return in_maps


def kernel(x, w_qkv, w_out, b_out):
    global _COMPILED
    x = np.asarray(x, np.float32)
    w_qkv = np.asarray(w_qkv, np.float32)
    w_out = np.asarray(w_out, np.float32)
    b_out = np.asarray(b_out, np.float32)

    if _COMPILED is None:
        _COMPILED = build_nc()
    nc = _COMPILED

    in_maps = _shard_inputs(x, w_qkv, w_out)
    res = run_bass_kernel_spmd(nc, in_maps, core_ids=list(range(8)))
    out = np.zeros((2, N_TOK, C), np.float32)
    for i in range(8):
        b = i // 4
        out[b] += res.results[i]["out"].astype(np.float32)
    out += b_out[None, None, :]
    return out


# revision 28
# speedup vs baseline: 1.0083x; 1.0083x over previous
"""Multi-head self-attention TRN2 Bass kernel (8-core SPMD).

Problem: x[2,2048,1024] -> qkv proj (w_qkv[1024,3072]) -> 16-head attention
-> out proj (w_out[1024,1024] + b_out) -> [2,2048,1024], all fp32.

Sharding: core i handles batch b=i//4 and head-group g=i%4 (4 heads each).
Each core computes a partial out-projection (its 256 rows of w_out); the
host sums the 4 partials per batch and adds the bias.

Per-core kernel layout strategy:
  - x arrives transposed from host as xT [c, tokens] fp16; DMAs split
    across the two HWDGE queues (sync + scalar), first-quarter columns
    first, so the first qkv chain starts after ~1.5MB instead of 4.5MB.
  - w_qkv slices arrive p-major (head-pair-major) so each head-pair's
    projection gates on 256KB.
  - qT/kT computed per head-pair as [128=2*64 d, 2048 tokens]; the k
    token-slices interleave with the first attention unit so the scalar
    engine (exp) starts as early as possible.
  - v computed in natural [keys, d] layout, 4 heads wide, stored with a
    constant-1.0 column block per head so the attention O^T matmul also
    produces the softmax denominators (partitions 64-127).
  - Attention per (head-pair p, q-slice of 512): for each key chunk kc,
    the TWO heads' score matmuls (K=64 row-tiles at partitions 0-63 /
    64-127) write adjacent 512-col halves of ONE PSUM tile [128,1024].
    Both matmuls gate on the same pool buffer, so the tile scheduler
    keeps them adjacent and the PE runs them CONCURRENTLY (row tiling,
    2x). One ACT exp (scale=1/8 fused) evicts the pair to SBUF fp16.
    O^T accumulates over 16 key-chunks with lhsT = v_aug.
    Normalize: DVE reciprocal of the sums rows, DVE multiply.
  - The last unit's O accumulator lives in the (by then idle) st pool so
    the final two units' O chains overlap instead of serializing on the
    small-psum pool.
  - Out projection partials written back fp16, summed on host in fp32.

Matmuls in fp16 (1 cycle/row on TRN2).
"""

import sys

if "/opt/trn_rl_repo" not in sys.path:
    sys.path.insert(0, "/opt/trn_rl_repo")

import numpy as np

import concourse.bacc as bacc
import concourse.mybir as mybir
import concourse.tile as tile
from concourse.bass_utils import run_bass_kernel_spmd

F32 = mybir.dt.float32
F16 = mybir.dt.float16

N_TOK = 2048
C = 1024
D = 64
CC = C // 128           # 8 contraction chunks
QS = N_TOK // 512       # 4 q-slices
KC = N_TOK // 128       # 16 key chunks

_COMPILED = None

MM_DT = F16


def build_nc(mm_dt=None):
    mm_dt = MM_DT if mm_dt is None else mm_dt
    nc = bacc.Bacc("TRN2", target_bir_lowering=False)

    xt_d = nc.declare_dram_parameter("xt", [C, N_TOK], mm_dt, isOutput=False)
    wq_d = nc.declare_dram_parameter("wq", [128, 2 * CC * 128], mm_dt, isOutput=False)
    wk_d = nc.declare_dram_parameter("wk", [128, 2 * CC * 128], mm_dt, isOutput=False)
    wv_d = nc.declare_dram_parameter("wv", [128, CC * 256], mm_dt, isOutput=False)
    wo_d = nc.declare_dram_parameter("wo", [128, 2 * C], mm_dt, isOutput=False)
    out_d = nc.declare_dram_parameter("out", [N_TOK, C], F16, isOutput=True)

    HP = CC * 128  # columns per head-pair in p-major weight layout

    with tile.TileContext(nc) as tc:
        with (
            tc.tile_pool(name="const", bufs=1) as const_pool,
            tc.tile_pool(name="qkt", bufs=1) as qkt_pool,
            tc.tile_pool(name="vsb", bufs=1) as v_pool,
            tc.tile_pool(name="otsb", bufs=1) as ot_sb_pool,
            tc.tile_pool(name="wop", bufs=1) as wo_pool,
            tc.tile_pool(name="outsb", bufs=2) as out_sb_pool,
            tc.tile_pool(name="pss", bufs=2, space="PSUM") as ps_small,
            tc.tile_pool(name="xTp", bufs=1) as xt_pool,
            tc.tile_pool(name="w3", bufs=3) as w3_pool,
            tc.tile_pool(name="stp", bufs=3, space="PSUM") as st_psum,
            tc.tile_pool(name="pt", bufs=36) as pt_pool,
            tc.tile_pool(name="rcp", bufs=2) as rcp_pool,
        ):
            wq_sb = w3_pool.tile([128, 2, CC, 128], mm_dt, tag="w3")
            wk_sb = w3_pool.tile([128, 2, CC, 128], mm_dt, tag="w3")
            wv_sb = w3_pool.tile([128, CC, 256], mm_dt, tag="w3")
            wo_sb = wo_pool.tile([128, 2, C], mm_dt, tag="wo")
            xT = xt_pool.tile([128, CC, N_TOK], mm_dt, tag="xT")

            # Input DMAs first, split across the two HWDGE queues (SP + ACT),
            # packed into few large transfers (engine DMA-issue costs ~600ns
            # each). The first k/q chains gate on the p=0 weight slices plus
            # the first-quarter token columns of every contraction chunk.
            wk_v = wk_sb[:].rearrange("p a c n -> p (a c n)")
            wq_v = wq_sb[:].rearrange("p a c n -> p (a c n)")
            nc.sync.dma_start(out=wk_v[:, 0:HP], in_=wk_d[:, 0:HP])
            nc.scalar.dma_start(out=wq_v[:, 0:HP], in_=wq_d[:, 0:HP])
            for cc in range(CC):
                eng = nc.sync if cc % 2 == 0 else nc.scalar
                eng.dma_start(out=xT[:, cc, 0:512], in_=xt_d[cc * 128:(cc + 1) * 128, 0:512])
            for cc in range(CC):
                eng = nc.sync if cc % 2 == 1 else nc.scalar
                eng.dma_start(out=xT[:, cc, 512:N_TOK], in_=xt_d[cc * 128:(cc + 1) * 128, 512:N_TOK])
            nc.sync.dma_start(out=wk_v[:, HP:2 * HP], in_=wk_d[:, HP:2 * HP])
            nc.scalar.dma_start(out=wq_v[:, HP:2 * HP], in_=wq_d[:, HP:2 * HP])
            nc.sync.dma_start(out=wv_sb[:].rearrange("p c n -> p (c n)"), in_=wv_d[:])
            nc.scalar.dma_start(out=wo_sb[:].rearrange("p c n -> p (c n)"), in_=wo_d[:])

            ones256 = const_pool.tile([128, 256], mm_dt, tag="ones256")
            nc.vector.memset(ones256[:], 1.0)
            # touch Exp early so the ACT table set loads during phase 1.
            actwarm = const_pool.tile([1, 1], F32, tag="actwarm")
            nc.scalar.activation(
                actwarm[:], ones256[0:1, 0:1], mybir.ActivationFunctionType.Exp
            )
            # PE warmup: no-dependency matmuls ramp the tensor-engine clock
            # (0.65 -> 2.4 GHz takes ~3us of busy) while input DMAs land, so
            # the first real qkv chains run at full speed.
            warm = st_psum.tile([128, 1024], F32, tag="st", name="warm")
            for i in range(12):
                nc.tensor.matmul(
                    warm[:, 0:256],
                    ones256[:, 0:128],
                    ones256[:],
                    start=True,
                    stop=True,
                )

            qT = [qkt_pool.tile([128, N_TOK], mm_dt, tag=f"qT{p}", name=f"qT{p}") for p in range(2)]
            kT = [qkt_pool.tile([128, N_TOK], mm_dt, tag=f"kT{p}", name=f"kT{p}") for p in range(2)]
            oT = [ot_sb_pool.tile([128, N_TOK], mm_dt, tag=f"oT{p}", name=f"oT{p}") for p in range(2)]
            # v_aug: per key-chunk, per head: [64 v columns | 64 ones columns]
            # -> the O^T matmul (M=128) emits softmax denominators on
            # partitions 64..127, already broadcast.
            v_sb = v_pool.tile([128, KC, 4 * 128], mm_dt, tag="v")
            v_aug_view = v_sb[:].rearrange("p k (h e) -> p k h e", h=4)

            def emit_k_ts(p, ts):
                sl = slice(ts * 512, (ts + 1) * 512)
                acc = ps_small.tile([128, 512], F32, tag="ps", name="acc")
                for cc in range(CC):
                    nc.tensor.matmul(
                        acc[:],
                        wk_sb[:, p, cc, :],
                        xT[:, cc, sl],
                        start=(cc == 0),
                        stop=(cc == CC - 1),
                    )
                nc.vector.tensor_copy(kT[p][:, sl], acc[:])

            def emit_q(p, ts):
                sl = slice(ts * 512, (ts + 1) * 512)
                acc = ps_small.tile([128, 512], F32, tag="ps", name="acc")
                for cc in range(CC):
                    nc.tensor.matmul(
                        acc[:],
                        wq_sb[:, p, cc, :],
                        xT[:, cc, sl],
                        start=(cc == 0),
                        stop=(cc == CC - 1),
                    )
                nc.vector.tensor_copy(qT[p][:, sl], acc[:])

            def emit_v(lo, hi):
                for kc in range(lo, hi):
                    acc = ps_small.tile([128, 256], F32, tag="ps", name="acc")
                    for cc in range(CC):
                        nc.tensor.matmul(
                            acc[:],
                            xT[:, cc, kc * 128:(kc + 1) * 128],
                            wv_sb[:, cc, :],
                            start=(cc == 0),
                            stop=(cc == CC - 1),
                        )
                    nc.vector.tensor_copy(
                        v_aug_view[:, kc, :, 0:64],
                        acc[:].rearrange("p (h e) -> p h e", h=4),
                    )

            pts_map = {}
            ots_map = {}

            def emit_sexp(ts, p, k0, k1):
                # Per key-chunk: both heads' score matmuls (K=64 row tiles at
                # partitions 0-63 / 64-127) write adjacent halves of one PSUM
                # tile -> PE runs them concurrently. Bursts of 2 kc reduce
                # PE array mode switches (64-row tiling <-> full 128).
                sl = slice(ts * 512, (ts + 1) * 512)
                unit = pts_map.setdefault((ts, p), {})
                for kc0 in range(k0, k1, 2):
                    sts = []
                    for kc in (kc0, kc0 + 1):
                        st = st_psum.tile([128, 1024], F32, tag="st", name="st")
                        for h in range(2):
                            hp = slice(h * 64, (h + 1) * 64)
                            nc.tensor.matmul(
                                st[:, h * 512:(h + 1) * 512],
                                kT[p][hp, kc * 128:(kc + 1) * 128],
                                qT[p][hp, sl],
                                start=True,
                                stop=True,
                            )
                        sts.append(st)
                    for st, kc in zip(sts, (kc0, kc0 + 1)):
                        pt = pt_pool.tile([128, 1024], mm_dt, tag="pt", name="pt")
                        nc.scalar.activation(
                            pt[:],
                            st[:],
                            mybir.ActivationFunctionType.Exp,
                            scale=0.125,
                        )
                        unit[kc] = pt

            def emit_o(ts, p, k0, k1, act_norm=False):
                unit = pts_map[(ts, p)]
                if (ts, p) not in ots_map:
                    ots_map[(ts, p)] = [
                        ps_small.tile([128, 512], F32, tag="ps", name=f"ot{h}")[:]
                        for h in range(2)
                    ]
                ots = ots_map[(ts, p)]
                for kc in range(k0, k1):
                    pt = unit.pop(kc)
                    for h in range(2):
                        a = 2 * p + h
                        nc.tensor.matmul(
                            ots[h],
                            v_sb[:, kc, a * 128:(a + 1) * 128],
                            pt[:, h * 512:(h + 1) * 512],
                            start=(kc == 0),
                            stop=(kc == KC - 1),
                        )
                if k1 < KC:
                    return
                sl = slice(ts * 512, (ts + 1) * 512)
                numer = rcp_pool.tile([128, 512], F32, tag="numer")
                sums = rcp_pool.tile([128, 512], F32, tag="sums")
                for h in range(2):
                    hq = slice(h * 64, (h + 1) * 64)
                    nc.vector.tensor_copy(numer[hq, :], ots[h][0:64, :])
                    if act_norm:
                        # last unit: scalar engine is idle by now; moving the
                        # sums eviction there shortens the tail's DVE chain.
                        nc.scalar.activation(
                            sums[hq, :], ots[h][64:128, :],
                            mybir.ActivationFunctionType.Copy,
                        )
                    else:
                        nc.vector.tensor_copy(sums[hq, :], ots[h][64:128, :])
                rcp = rcp_pool.tile([128, 512], F32, tag="rcp")
                scr = rcp_pool.tile([128, 512], F32, tag="scr")
                nc.vector.reciprocal_approx_accurate(
                    out=rcp[:], in_=sums[:], scratch=scr[:]
                )
                nc.vector.tensor_mul(oT[p][:, sl], numer[:], rcp[:])

            def emit_proj(t0, t1, split_dma=False, act_evict=False):
                # proj accumulators live in the st tag so proj bursts do not
                # touch the ps rotation (which sequences the O accumulators).
                for t in range(t0, t1):
                    outp = out_sb_pool.tile([128, C], F16, tag="outp")
                    for ns in range(2):
                        po = st_psum.tile([128, 512], F32, tag="st", name="po")
                        for p in range(2):
                            nc.tensor.matmul(
                                po[:],
                                oT[p][:, t * 128:(t + 1) * 128],
                                wo_sb[:, p, ns * 512:(ns + 1) * 512],
                                start=(p == 0),
                                stop=(p == 1),
                            )
                        if act_evict and ns == 1:
                            # tail: scalar engine is idle after the last exp;
                            # same f32-PSUM -> f16-SBUF path the exp uses.
                            nc.scalar.activation(
                                outp[:, ns * 512:(ns + 1) * 512], po[:],
                                mybir.ActivationFunctionType.Copy,
                            )
                        else:
                            nc.vector.tensor_copy(outp[:, ns * 512:(ns + 1) * 512], po[:])
                    eng = nc.scalar if (split_dma and t % 2) else nc.sync
                    eng.dma_start(
                        out=out_d[t * 128:(t + 1) * 128, :], in_=outp[:]
                    )

            # Software-pipelined schedule. Unit order:
            #   u0..u7 = (0,0),(1,0),(2,0),(0,1),(1,1),(3,0),(2,1),(3,1)
            # Each unit window weaves 2-kc sexp batches (the ACT pacer) with
            # small slices of the previous unit's O accumulation and PE
            # fillers (v, k/q chains, proj t-chunks) so the PE keeps the
            # scalar engine fed instead of draining whole blocks first.
            # The last three units' O chase their own exps (the ps rotation
            # frees up once qkv accs are done) to shrink the tail.
            # opening + u0 = (0,0)
            emit_k_ts(0, 0)
            emit_q(0, 0)
            emit_sexp(0, 0, 0, 4)
            emit_k_ts(0, 1)
            emit_sexp(0, 0, 4, 8)
            emit_k_ts(0, 2)
            emit_sexp(0, 0, 8, 12)
            emit_k_ts(0, 3)
            emit_sexp(0, 0, 12, 14)
            emit_q(0, 1)
            for kc in range(KC):
                nc.vector.tensor_copy(
                    v_aug_view[:, kc, :, 64:128],
                    ones256[:].rearrange("p (h e) -> p h e", h=4),
                )
            emit_sexp(1, 0, 0, 2)               # handover u0 -> u1
            emit_sexp(0, 0, 14, 16); emit_v(0, 4)
            # u1 = (1,0)
            emit_sexp(1, 0, 2, 4); emit_v(4, 6)
            emit_sexp(1, 0, 4, 6); emit_v(6, 8)
            emit_sexp(1, 0, 6, 8); emit_v(8, 10)
            emit_sexp(1, 0, 8, 10); emit_v(10, 12)
            emit_sexp(1, 0, 10, 12); emit_v(12, 14)
            emit_sexp(1, 0, 12, 14); emit_v(14, 16); emit_q(0, 2)
            emit_sexp(2, 0, 0, 2)               # handover u1 -> u2
            emit_sexp(1, 0, 14, 16)
            # u2 = (2,0)
            emit_sexp(2, 0, 2, 4); emit_o(0, 0, 0, 4)
            emit_sexp(2, 0, 4, 6); emit_o(0, 0, 4, 8)
            emit_sexp(2, 0, 6, 8); emit_o(0, 0, 8, 12)
            emit_sexp(2, 0, 8, 10); emit_o(0, 0, 12, 16)
            emit_sexp(2, 0, 10, 12); emit_k_ts(1, 0)
            emit_sexp(2, 0, 12, 14); emit_q(1, 0); emit_k_ts(1, 1)
            emit_sexp(0, 1, 0, 2)               # handover u2 -> u3
            emit_sexp(2, 0, 14, 16); emit_k_ts(1, 2)
            # u3 = (0,1)
            emit_sexp(0, 1, 2, 4); emit_k_ts(1, 3)
            emit_sexp(0, 1, 4, 6); emit_q(1, 1)
            emit_sexp(0, 1, 6, 8); emit_o(1, 0, 0, 4)
            emit_sexp(0, 1, 8, 10); emit_o(1, 0, 4, 8)
            emit_sexp(0, 1, 10, 12); emit_o(1, 0, 8, 12)
            emit_sexp(0, 1, 12, 14); emit_o(1, 0, 12, 16)
            emit_sexp(1, 1, 0, 2)               # handover u3 -> u4
            emit_sexp(0, 1, 14, 16); emit_q(0, 3)
            # u4 = (1,1)
            emit_sexp(1, 1, 2, 4); emit_o(2, 0, 0, 4)
            emit_sexp(1, 1, 4, 6); emit_o(2, 0, 4, 8)
            emit_sexp(1, 1, 6, 8); emit_o(2, 0, 8, 12)
            emit_sexp(1, 1, 8, 10); emit_o(2, 0, 12, 16)
            emit_sexp(1, 1, 10, 12); emit_o(0, 1, 0, 6)
            emit_sexp(1, 1, 12, 14); emit_o(0, 1, 6, 12)
            emit_sexp(3, 0, 0, 2)               # handover u4 -> u5
            emit_sexp(1, 1, 14, 16); emit_o(0, 1, 12, 16)
            # u5 = (3,0)
            emit_sexp(3, 0, 2, 4); emit_q(1, 2)
            emit_sexp(3, 0, 4, 6); emit_proj(0, 2)
            emit_sexp(3, 0, 6, 8); emit_proj(2, 4)
            emit_sexp(3, 0, 8, 10); emit_o(1, 1, 0, 6)
            emit_sexp(3, 0, 10, 12); emit_o(1, 1, 6, 12)
            emit_sexp(3, 0, 12, 14); emit_o(1, 1, 12, 16)
            emit_sexp(2, 1, 0, 2)               # handover u5 -> u6
            emit_sexp(3, 0, 14, 16); emit_q(1, 3)
            # u6 = (2,1)
            emit_sexp(2, 1, 2, 4); emit_proj(4, 6)
            emit_sexp(2, 1, 4, 6); emit_proj(6, 8)
            emit_sexp(2, 1, 6, 8); emit_o(3, 0, 0, 6)
            emit_sexp(2, 1, 8, 10); emit_o(3, 0, 6, 12)
            emit_sexp(2, 1, 10, 12); emit_o(3, 0, 12, 16)
            emit_sexp(2, 1, 12, 14); emit_o(2, 1, 0, 4)
            emit_sexp(3, 1, 0, 2)               # handover u6 -> u7
            emit_sexp(2, 1, 14, 16); emit_o(2, 1, 4, 6)
            # u7 = (3,1)
            emit_sexp(3, 1, 2, 4); emit_o(2, 1, 6, 10)
            emit_sexp(3, 1, 4, 6); emit_o(2, 1, 10, 14)
            emit_sexp(3, 1, 6, 8); emit_o(2, 1, 14, 16)
            emit_sexp(3, 1, 8, 10); emit_proj(8, 10)
            emit_sexp(3, 1, 10, 12); emit_proj(10, 12); emit_o(3, 1, 0, 4)
            emit_sexp(3, 1, 12, 14); emit_o(3, 1, 4, 8)
            emit_sexp(3, 1, 14, 16); emit_o(3, 1, 8, 12)
            emit_o(3, 1, 12, 16, act_norm=True)
            emit_proj(12, 16, split_dma=True, act_evict=True)

    nc.compile()
    return nc


def _shard_inputs(x, w_qkv, w_out):
    xts = [np.ascontiguousarray(x[b].T).astype(np.float16) for b in range(2)]
    in_maps = []
    for i in range(8):
        b, g = divmod(i, 4)
        cs = slice(256 * g, 256 * (g + 1))

        def shuf(w):
            # [1024, 256] -> [128, CC * 256] with chunk-major columns
            n = w.shape[1]
            return np.ascontiguousarray(
                w.reshape(CC, 128, n).transpose(1, 0, 2).reshape(128, CC * n)
            ).astype(np.float16)

        def shuf_p(w):
            # [1024, 256] -> [128, 2 * CC * 128] head-pair-major
            return np.ascontiguousarray(
                w.reshape(CC, 128, 2, 128).transpose(1, 2, 0, 3).reshape(128, 2 * CC * 128)
            ).astype(np.float16)

        in_maps.append({
            "xt": xts[b],
            "wq": shuf_p(w_qkv[:, cs]),
            "wk": shuf_p(w_qkv[:, 1024 + 256 * g:1024 + 256 * (g + 1)]),
            "wv": shuf(w_qkv[:, 2048 + 256 * g:2048 + 256 * (g + 1)]),
            "wo": np.ascontiguousarray(
                w_out[cs, :].reshape(2, 128, 1024).transpose(1, 0, 2).reshape(128, 2048)
            ).astype(np.float16),
        })
    return in_maps


def kernel(x, w_qkv, w_out, b_out):
    global _COMPILED
    x = np.asarray(x, np.float32)
    w_qkv = np.asarray(w_qkv, np.float32)
    w_out = np.asarray(w_out, np.float32)
    b_out = np.asarray(b_out, np.float32)

    if _COMPILED is None:
        _COMPILED = build_nc()
    nc = _COMPILED

    in_maps = _shard_inputs(x, w_qkv, w_out)
    res = run_bass_kernel_spmd(nc, in_maps, core_ids=list(range(8)))
    out = np.zeros((2, N_TOK, C), np.float32)
    for i in range(8):
        b = i // 4
        out[b] += res.results[i]["out"].astype(np.float32)
    out += b_out[None, None, :]
    return out
